# revision 27
# baseline (speedup 1.0000x reference)
"""Trainium2 Bass kernel for nn_LiveNet (2-layer MLP: relu(x@W1+b1)@W2+b2).

Sharding: pure data-parallel over batch across 8 NeuronCores (no
collectives).  Each core computes y_i = relu(x_i @ W1 + b1) @ W2 + b2 for
its 512-row batch shard.

Numerics: fp8 e4m3 DoubleRow matmuls (0.5 PE cycles per output row = 2x
the fp16 rate) with one-level error compensation.  Each operand A is split
host-side (or on-chip for h) into A_hi = fp8(A) and A_lo = fp8(A - A_hi)
at the SAME power-of-2 scale, and each GEMM runs three fp8 streams

    A_hi@B_hi + A_lo@B_hi + A_hi@B_lo  ~=  A@B   (drops the lo*lo term)

accumulated in one fp32 PSUM group.  Error ~0.2% rel (vs 5% for plain
fp8, 0.04% for fp16) at 1.5x the DoubleRow unit cost = 0.75x the fp16 PE
time: the PE floor drops from 109.2us to 81.9us.  On top of that, GEMM2's
compensation streams cover only a k-prefix (HLO_KTS/W2LO_KTS of 16
k-steps), spending error budget (0.18% -> measured 1.41% vs the 2e-2
gate) for another 4.3us of PE time.

Scales (all powers of 2, folded into the host-side prep):
  x at 1, W1 at 32  -> PSUM1 = 32*(x@W1); b1 pre-scaled by 32.
  h kept at scale 32 (fp16 intermediate + fp8 hi/lo), W2 at 2048
  -> PSUM2 = 65536*y; eviction computes (ps + 65536*b2) * 2^-16.

DoubleRow layout: contraction index k = 256*kt + 128*i + p, where p is
the SBUF partition and i the 2-plane dim; lhsT tiles are [128p, 2i, 128m],
moving tiles [128p, 2i, 512n], PSUM out [128m, 512n] per instruction
(256 cycles each at full clock).

Per-core dataflow:
  Warmup: junk fp16 matmuls burn the 3us p-state ramp while startup DMAs
          are in flight (the ramp resets if the PE idles, so junk must
          bridge exactly to the first operand arrival at ~2.4us).
  Queues: DMAs hold their issuing queue >=500ns each (descriptor-gen
          floor), so startup fans the first 10 operand tiles over all
          three DMA-capable queues (SP / ACT / Pool) in m=0's consumption
          order.  Steady state: SP carries the W1 hi/lo stream + biases,
          Pool carries the W2 hi/lo stream and y stores.
  GEMM1:  per hid m-tile: 12 DoubleRow matmuls (3 streams x 4 k-steps)
          -> PSUM [128,512]; ACT evicts h16 = relu(ps + 32*b1) (fp16),
          h_hi = fp8(h16) alternates ACT/DVE, DVE computes
          h_lo = fp8(h16 - h_hi).  h_hi/h_lo land in the resident ht
          tile laid out for GEMM2.
  GEMM2:  per out-col tile (128 cols): 43 DoubleRow matmuls (16 + 13 + 14
          k-steps over the 3 streams), lhsT = W2 slices (resident),
          moving = ht slices; DVE evicts y = (ps + 65536*b2)*2^-16; Pool
          stores the yT chunk (last chunk on SP - it is the makespan
          tail, so the last tile is split 256/128/64/64 over batch).
          Output is produced transposed (yT [1024, 512]); host transposes.
"""

import os
import sys

import numpy as np
import ml_dtypes

for _p in ("/opt/trn_rl_repo", "/root/.axon_site/_ro/trn_rl_repo"):
    if os.path.isdir(_p) and _p not in sys.path:
        sys.path.append(_p)

import concourse.bacc as bacc
import concourse.bass as bass
import concourse.tile as tile
from concourse import mybir
from concourse.bass_utils import run_bass_kernel_spmd

N_CORES = 8
B, N_IN, N_HID, N_OUT = 4096, 1024, 4096, 1024
BSH = B // N_CORES          # 512 batch rows per core
P = 128                     # SBUF partitions
KT1 = N_IN // (2 * P)       # 4  DoubleRow k-steps in GEMM1
MT1 = N_HID // P            # 32 hid m-tiles in GEMM1
KT2 = N_HID // (2 * P)      # 16 DoubleRow k-steps in GEMM2
NT2 = N_OUT // P            # 8  out-col tiles in GEMM2
N_WARM = 19                 # junk warmup matmuls (bridge to first operand)

SW1 = 32.0                  # W1 scale; also the h scale (x scale is 1)
SW2 = 2048.0                # W2 scale
SY = float(2.0 ** -16)      # output descale (1/(32*2048))

# Error-budget spend: GEMM2's compensation streams cover only a k-prefix
# (h_lo: kts [0,13), w2_lo: kts [0,14) of 16).  Dropping the tail k-steps
# of the residual streams raises rel err from 0.18% to a measured 1.41%
# (gate 2e-2) and saves 5 x 0.85us of PE time.
HLO_KTS = 13
W2LO_KTS = 14

F32 = mybir.dt.float32
F16 = mybir.dt.float16
F8 = mybir.dt.float8e4
DR = mybir.MatmulPerfMode.DoubleRow
AF = mybir.ActivationFunctionType
OP = mybir.AluOpType
E4 = ml_dtypes.float8_e4m3


def build_nc():
    nc = bacc.Bacc("TRN2", target_bir_lowering=False, debug=False,
                   num_devices=N_CORES)

    xt8 = nc.declare_dram_parameter("xt8", [2, KT1, P, 2, BSH], F8,
                                    isOutput=False)
    w1q = nc.declare_dram_parameter("w1q", [MT1, 2, P, KT1, 2, P], F8,
                                    isOutput=False)
    w2q = nc.declare_dram_parameter("w2q", [2, KT2, P, 2, N_OUT], F8,
                                    isOutput=False)
    b1t = nc.declare_dram_parameter("b1t", [P, MT1], F32, isOutput=False)
    b2t = nc.declare_dram_parameter("b2t", [P, NT2], F32, isOutput=False)
    yt = nc.declare_dram_parameter("yt", [N_OUT, BSH], F32, isOutput=True)

    with tile.TileContext(nc) as tc:
        with (
            tc.tile_pool(name="const", bufs=1) as const,
            tc.tile_pool(name="xt", bufs=1) as xt_pool,
            tc.tile_pool(name="w1", bufs=8) as w1_pool,
            tc.tile_pool(name="big", bufs=1) as big_pool,
            tc.tile_pool(name="h16", bufs=4) as h16_pool,
            tc.tile_pool(name="yout", bufs=4) as y_pool,
            tc.tile_pool(name="ps", bufs=8, space=bass.MemorySpace.PSUM) as ps_pool,
        ):
            # PE warmup on a zeroed tile: keeps the PE busy from ~0.1us so
            # the p-state ramp overlaps the startup DMA latency (the ramp
            # resets if the PE goes idle, so warmup must bridge the gap to
            # the first operand arrival at ~2.3us).
            junk = const.tile([P, P], F16)
            nc.vector.memset(junk[:], 0.0)
            ps_junk = ps_pool.tile([P, P], F32, tag="ps", name="ps_junk")
            for j in range(N_WARM):
                nc.tensor.matmul(
                    ps_junk[:], junk[:], junk[:],
                    start=(j == 0), stop=(j == N_WARM - 1),
                )

            # x hi/lo tiles, persistent: [128p, 2i, 512n] per (s, kt).
            xt_sb = [
                [
                    xt_pool.tile([P, 2, BSH], F8, tag=f"x{s}{k}",
                                 name=f"x{s}{k}")
                    for k in range(KT1)
                ]
                for s in range(2)
            ]

            w1_tiles = {}

            def w1_load(m, eng=None):
                th = w1_pool.tile([P, KT1, 2, P], F8, tag="w1",
                                  name=f"w1h{m}")
                (eng or nc.sync).dma_start(out=th[:], in_=w1q[m, 0])
                tl = w1_pool.tile([P, KT1, 2, P], F8, tag="w1",
                                  name=f"w1l{m}")
                (eng or nc.sync).dma_start(out=tl[:], in_=w1q[m, 1])
                w1_tiles[m] = (th, tl)

            # Startup DMAs fanned over the THREE DMA-capable queues (each
            # DMA holds its queue >=500ns, so two queues cannot feed m=0's
            # cadence).  ACT is idle until the first eviction (~4.5us) and
            # carries early x tiles.  Completion ~= 500*(slot+1) + 1716:
            #   SP:   w1h0 xh3 xl1 w1h1 b1 w1l1 w1h2 w1l2 | loop prefetch
            #   ACT:  xh1  xh2 xl2 w1l0 b2
            #   Pool: xh0  xl0 xl3 | w2-stream...
            th0 = w1_pool.tile([P, KT1, 2, P], F8, tag="w1", name="w1h0")
            tl0 = w1_pool.tile([P, KT1, 2, P], F8, tag="w1", name="w1l0")
            w1_tiles[0] = (th0, tl0)
            b1_sb = const.tile([P, MT1], F32)
            b2_sb = const.tile([P, NT2], F32)
            th1 = w1_pool.tile([P, KT1, 2, P], F8, tag="w1", name="w1h1")
            tl1 = w1_pool.tile([P, KT1, 2, P], F8, tag="w1", name="w1l1")
            w1_tiles[1] = (th1, tl1)
            nc.sync.dma_start(out=th0[:], in_=w1q[0, 0])
            nc.scalar.dma_start(out=xt_sb[0][1][:], in_=xt8[0, 1])
            nc.gpsimd.dma_start(out=xt_sb[0][0][:], in_=xt8[0, 0])
            nc.sync.dma_start(out=xt_sb[0][3][:], in_=xt8[0, 3])
            nc.scalar.dma_start(out=xt_sb[0][2][:], in_=xt8[0, 2])
            nc.gpsimd.dma_start(out=xt_sb[1][0][:], in_=xt8[1, 0])
            nc.sync.dma_start(out=xt_sb[1][1][:], in_=xt8[1, 1])
            nc.scalar.dma_start(out=xt_sb[1][2][:], in_=xt8[1, 2])
            nc.gpsimd.dma_start(out=xt_sb[1][3][:], in_=xt8[1, 3])
            nc.sync.dma_start(out=th1[:], in_=w1q[1, 0])
            nc.scalar.dma_start(out=tl0[:], in_=w1q[0, 1])
            nc.sync.dma_start(out=b1_sb[:], in_=b1t[:])
            nc.scalar.dma_start(out=b2_sb[:], in_=b2t[:])
            nc.sync.dma_start(out=tl1[:], in_=w1q[1, 1])
            w1_load(2)
            # kt consumption order per stream, matched to arrival order.
            KORD = [[1, 0, 3, 2], [0, 1, 2, 3], [0, 1, 2, 3]]

            # Prime ACT/DVE with the bias-load DMA waits so later evicts
            # (which also wait on PE sems) stay within the per-instruction
            # sync-wait budget in walrus codegen.
            prime1 = const.tile([P, 1], F32)
            nc.scalar.activation(prime1[:], b1_sb[:, 0:1], AF.Copy)
            prime2 = const.tile([P, 1], F32)
            nc.vector.tensor_copy(prime2[:], b2_sb[:, 0:1])

            # Resident tiles: W2 hi/lo [128, 2s, 16kt, 2i, 1024] (8MB) and
            # hT hi/lo [128, 2s, 16kt, 2i, 512] (4MB).
            w2_sb = big_pool.tile([P, 2, KT2, 2, N_OUT], F8, tag="w2",
                                  name="w2_sb")
            ht_sb = big_pool.tile([P, 2, KT2, 2, BSH], F8, tag="ht",
                                  name="ht_sb")

            # ---- GEMM1: h = relu(x @ W1 + b1), 3-stream fp8 DoubleRow ----
            for m in range(MT1):
                if m + 3 < MT1:
                    w1_load(m + 3)
                # Pace the W2 stream at one (s, kt) slice per m-iter on the
                # Pool queue: hi slices on m=0..15, lo on m=16.. (only the
                # k-prefix the w2_lo stream reads).
                if m < KT2 + W2LO_KTS:
                    ws, wkt = (0, m) if m < KT2 else (1, m - KT2)
                    nc.gpsimd.dma_start(out=w2_sb[:, ws, wkt, :, :],
                                        in_=w2q[ws, wkt])

                ps = ps_pool.tile([P, BSH], F32, tag="ps", name="ps")
                th, tl = w1_tiles.pop(m)
                nmm = 0
                for si, (wt, xs) in enumerate(((th, 0), (th, 1), (tl, 0))):
                    for kt in (KORD[si] if m == 0 else range(KT1)):
                        nc.tensor.matmul(
                            ps[:],
                            wt[:, kt, :, :],
                            xt_sb[xs][kt][:],
                            start=(nmm == 0),
                            stop=(nmm == 3 * KT1 - 1),
                            perf_mode=DR,
                        )
                        nmm += 1

                kt2, pl = m // 2, m % 2
                h16 = h16_pool.tile([P, BSH], F16, tag="h16", name="h16")
                nc.scalar.activation(h16[:], ps[:], AF.Relu,
                                     bias=b1_sb[:, m:m + 1], scale=1.0)
                hhi = ht_sb[:, 0, kt2, pl, :]
                # Alternate the h_hi cast between ACT and DVE so neither
                # engine saturates the GEMM1 window.
                if m % 2 == 0:
                    nc.scalar.activation(hhi, h16[:], AF.Copy)
                else:
                    nc.vector.tensor_copy(hhi, h16[:])
                if kt2 < HLO_KTS:
                    nc.vector.scalar_tensor_tensor(
                        ht_sb[:, 1, kt2, pl, :], h16[:], 1.0, hhi,
                        OP.mult, OP.subtract,
                    )

            # ---- GEMM2: yT = (hT.T-tiled @ W2 slices) + b2, 3-stream ----
            # The last tile is split into batch sub-chunks so the
            # after-last-matmul tail is a small evict + store instead of a
            # full [128, 512] evict + 256KB DMA.
            chunks = []
            for t in range(NT2):
                if t == NT2 - 1:
                    for c0, cw in ((0, 384), (384, 128)):
                        chunks.append((t, c0, cw))
                else:
                    chunks.append((t, 0, BSH))
            g2mms = []
            for kt in range(KT2):
                g2mms.append((0, 0, kt))
                if kt < HLO_KTS:
                    g2mms.append((0, 1, kt))
                if kt < W2LO_KTS:
                    g2mms.append((1, 0, kt))
            for t, c0, cw in chunks:
                ps2 = ps_pool.tile([P, cw], F32, tag="ps", name="ps2")
                for nmm, (ws, hs, kt) in enumerate(g2mms):
                    nc.tensor.matmul(
                        ps2[:],
                        w2_sb[:, ws, kt, :, t * P:(t + 1) * P],
                        ht_sb[:, hs, kt, :, c0:c0 + cw],
                        start=(nmm == 0),
                        stop=(nmm == len(g2mms) - 1),
                        perf_mode=DR,
                    )
                y_sb = y_pool.tile([P, cw], F32, tag="y", name="y_sb")
                nc.vector.tensor_scalar(
                    y_sb[:], ps2[:], b2_sb[:, t:t + 1], SY,
                    OP.add, OP.mult,
                )
                # The final store goes on SP (shorter DMA init than Pool —
                # it is the makespan tail).
                eng = nc.sync if (t, c0) == (NT2 - 1, 384) else nc.gpsimd
                eng.dma_start(out=yt[t * P:(t + 1) * P, c0:c0 + cw],
                              in_=y_sb[:])
    nc.compile()
    return nc


def _split8(a):
    hi = a.astype(E4)
    lo = (a - hi.astype(np.float32)).astype(E4)
    return hi, lo


def _prep_shared(W1, b1, W2, b2):
    W1s = np.asarray(W1, np.float32) * SW1          # [1024, 4096]
    w1hi, w1lo = _split8(W1s)
    a = np.stack([w1hi, w1lo])                      # [2s, 1024, 4096]
    a = a.reshape(2, KT1, 2, P, MT1, P)             # [s, kt, i, p, m, c]
    w1q = np.ascontiguousarray(a.transpose(4, 0, 3, 1, 2, 5))

    W2s = np.asarray(W2, np.float32) * SW2          # [4096, 1024]
    w2hi, w2lo = _split8(W2s)
    b = np.stack([w2hi, w2lo])                      # [2s, 4096, 1024]
    b = b.reshape(2, KT2, 2, P, N_OUT)              # [s, kt, i, p, c]
    w2q = np.ascontiguousarray(b.transpose(0, 1, 3, 2, 4))

    b1t = np.ascontiguousarray(
        (np.asarray(b1, np.float32) * SW1).reshape(MT1, P).T)
    b2t = np.ascontiguousarray(
        (np.asarray(b2, np.float32) / SY).reshape(NT2, P).T)
    return w1q, b1t, w2q, b2t


def kernel(x, W1, b1, W2, b2):
    x = np.asarray(x, np.float32)
    w1q, b1t, w2q, b2t = _prep_shared(W1, b1, W2, b2)

    in_maps = []
    for i in range(N_CORES):
        xT = np.ascontiguousarray(x[i * BSH:(i + 1) * BSH, :].T)
        xhi, xlo = _split8(xT)                      # [1024, 512] each
        a = np.stack([xhi, xlo]).reshape(2, KT1, 2, P, BSH)
        xt8 = np.ascontiguousarray(a.transpose(0, 1, 3, 2, 4))
        in_maps.append(
            {"xt8": xt8, "w1q": w1q, "w2q": w2q, "b1t": b1t, "b2t": b2t}
        )

    nc = build_nc()
    res = run_bass_kernel_spmd(nc, in_maps, list(range(N_CORES)))
    y = np.concatenate(
        [np.asarray(res.results[i]["yt"], dtype=np.float32).T
         for i in range(N_CORES)],
        axis=0,
    )
    return np.ascontiguousarray(y)


if __name__ == "__main__":
    rng = np.random.default_rng(0)
    x = rng.standard_normal((B, N_IN), dtype=np.float32)
    W1 = rng.standard_normal((N_IN, N_HID), dtype=np.float32) / 32
    b1 = rng.standard_normal((N_HID,), dtype=np.float32) / 32
    W2 = rng.standard_normal((N_HID, N_OUT), dtype=np.float32) / 64
    b2 = rng.standard_normal((N_OUT,), dtype=np.float32) / 64
    y = kernel(x, W1, b1, W2, b2)
    h = np.maximum(x @ W1 + b1, 0)
    y_ref = h @ W2 + b2
    err = np.linalg.norm(y - y_ref) / np.linalg.norm(y_ref)
    print("rel_l2:", err)


# revision 28
# speedup vs baseline: 1.0013x; 1.0013x over previous
"""Trainium2 Bass kernel for nn_LiveNet (2-layer MLP: relu(x@W1+b1)@W2+b2).

Sharding: pure data-parallel over batch across 8 NeuronCores (no
collectives).  Each core computes y_i = relu(x_i @ W1 + b1) @ W2 + b2 for
its 512-row batch shard.

Numerics: fp8 e4m3 DoubleRow matmuls (0.5 PE cycles per output row = 2x
the fp16 rate) with one-level error compensation.  Each operand A is split
host-side (or on-chip for h) into A_hi = fp8(A) and A_lo = fp8(A - A_hi)
at the SAME power-of-2 scale, and each GEMM runs three fp8 streams

    A_hi@B_hi + A_lo@B_hi + A_hi@B_lo  ~=  A@B   (drops the lo*lo term)

accumulated in one fp32 PSUM group.  Error ~0.2% rel (vs 5% for plain
fp8, 0.04% for fp16) at 1.5x the DoubleRow unit cost = 0.75x the fp16 PE
time: the PE floor drops from 109.2us to 81.9us.  On top of that, GEMM2's
compensation streams cover only a k-prefix (HLO_KTS/W2LO_KTS of 16
k-steps), spending error budget (0.18% -> measured 1.41% vs the 2e-2
gate) for another 4.3us of PE time.

Scales (all powers of 2, folded into the host-side prep):
  x at 1, W1 at 32  -> PSUM1 = 32*(x@W1); b1 pre-scaled by 32.
  h kept at scale 32 (fp16 intermediate + fp8 hi/lo), W2 at 2048
  -> PSUM2 = 65536*y; eviction computes (ps + 65536*b2) * 2^-16.

DoubleRow layout: contraction index k = 256*kt + 128*i + p, where p is
the SBUF partition and i the 2-plane dim; lhsT tiles are [128p, 2i, 128m],
moving tiles [128p, 2i, 512n], PSUM out [128m, 512n] per instruction
(256 cycles each at full clock).

Per-core dataflow:
  Warmup: junk fp16 matmuls burn the 3us p-state ramp while startup DMAs
          are in flight (the ramp resets if the PE idles, so junk must
          bridge exactly to the first operand arrival at ~2.4us).
  Queues: DMAs hold their issuing queue >=500ns each (descriptor-gen
          floor), so startup fans the first 10 operand tiles over all
          three DMA-capable queues (SP / ACT / Pool) in m=0's consumption
          order.  Steady state: SP carries the W1 hi/lo stream + biases,
          Pool carries the W2 hi/lo stream and y stores.
  GEMM1:  per hid m-tile: 12 DoubleRow matmuls (3 streams x 4 k-steps)
          -> PSUM [128,512]; ACT evicts h16 = relu(ps + 32*b1) (fp16),
          h_hi = fp8(h16) alternates ACT/DVE, DVE computes
          h_lo = fp8(h16 - h_hi).  h_hi/h_lo land in the resident ht
          tile laid out for GEMM2.
  GEMM2:  per out-col tile (128 cols): 43 DoubleRow matmuls (16 + 13 + 14
          k-steps over the 3 streams), lhsT = W2 slices (resident),
          moving = ht slices; DVE evicts y = (ps + 65536*b2)*2^-16; Pool
          stores the yT chunk (last chunk on SP - it is the makespan
          tail, so the last tile is split 256/128/64/64 over batch).
          Output is produced transposed (yT [1024, 512]); host transposes.
"""

import os
import sys

import numpy as np
import ml_dtypes

for _p in ("/opt/trn_rl_repo", "/root/.axon_site/_ro/trn_rl_repo"):
    if os.path.isdir(_p) and _p not in sys.path:
        sys.path.append(_p)

import concourse.bacc as bacc
import concourse.bass as bass
import concourse.tile as tile
from concourse import mybir
from concourse.bass_utils import run_bass_kernel_spmd

N_CORES = 8
B, N_IN, N_HID, N_OUT = 4096, 1024, 4096, 1024
BSH = B // N_CORES          # 512 batch rows per core
P = 128                     # SBUF partitions
KT1 = N_IN // (2 * P)       # 4  DoubleRow k-steps in GEMM1
MT1 = N_HID // P            # 32 hid m-tiles in GEMM1
KT2 = N_HID // (2 * P)      # 16 DoubleRow k-steps in GEMM2
NT2 = N_OUT // P            # 8  out-col tiles in GEMM2
N_WARM = 19                 # junk warmup matmuls (bridge to first operand)

SW1 = 32.0                  # W1 scale; also the h scale (x scale is 1)
SW2 = 2048.0                # W2 scale
SY = float(2.0 ** -16)      # output descale (1/(32*2048))

# Error-budget spend: GEMM2's compensation streams cover only a k-prefix
# (h_lo: kts [0,13), w2_lo: kts [0,14) of 16).  Dropping the tail k-steps
# of the residual streams raises rel err from 0.18% to a measured 1.41%
# (gate 2e-2) and saves 5 x 0.85us of PE time.
HLO_KTS = 13
W2LO_KTS = 14

F32 = mybir.dt.float32
F16 = mybir.dt.float16
F8 = mybir.dt.float8e4
DR = mybir.MatmulPerfMode.DoubleRow
AF = mybir.ActivationFunctionType
OP = mybir.AluOpType
E4 = ml_dtypes.float8_e4m3


def build_nc():
    nc = bacc.Bacc("TRN2", target_bir_lowering=False, debug=False,
                   num_devices=N_CORES)

    xt8 = nc.declare_dram_parameter("xt8", [2, KT1, P, 2, BSH], F8,
                                    isOutput=False)
    w1q = nc.declare_dram_parameter("w1q", [MT1, 2, P, KT1, 2, P], F8,
                                    isOutput=False)
    w2q = nc.declare_dram_parameter("w2q", [2, KT2, P, 2, N_OUT], F8,
                                    isOutput=False)
    b1t = nc.declare_dram_parameter("b1t", [P, MT1], F32, isOutput=False)
    b2t = nc.declare_dram_parameter("b2t", [P, NT2], F32, isOutput=False)
    yt = nc.declare_dram_parameter("yt", [N_OUT, BSH], F32, isOutput=True)

    with tile.TileContext(nc) as tc:
        with (
            tc.tile_pool(name="const", bufs=1) as const,
            tc.tile_pool(name="xt", bufs=1) as xt_pool,
            tc.tile_pool(name="w1", bufs=8) as w1_pool,
            tc.tile_pool(name="big", bufs=1) as big_pool,
            tc.tile_pool(name="h16", bufs=4) as h16_pool,
            tc.tile_pool(name="yout", bufs=4) as y_pool,
            tc.tile_pool(name="ps", bufs=8, space=bass.MemorySpace.PSUM) as ps_pool,
        ):
            # PE warmup on a zeroed tile: keeps the PE busy from ~0.1us so
            # the p-state ramp overlaps the startup DMA latency (the ramp
            # resets if the PE goes idle, so warmup must bridge the gap to
            # the first operand arrival at ~2.3us).
            junk = const.tile([P, P], F16)
            nc.vector.memset(junk[:], 0.0)
            ps_junk = ps_pool.tile([P, P], F32, tag="ps", name="ps_junk")
            for j in range(N_WARM):
                nc.tensor.matmul(
                    ps_junk[:], junk[:], junk[:],
                    start=(j == 0), stop=(j == N_WARM - 1),
                )

            # x hi/lo tiles, persistent: [128p, 2i, 512n] per (s, kt).
            xt_sb = [
                [
                    xt_pool.tile([P, 2, BSH], F8, tag=f"x{s}{k}",
                                 name=f"x{s}{k}")
                    for k in range(KT1)
                ]
                for s in range(2)
            ]

            w1_tiles = {}

            def w1_load(m, eng=None):
                th = w1_pool.tile([P, KT1, 2, P], F8, tag="w1",
                                  name=f"w1h{m}")
                (eng or nc.sync).dma_start(out=th[:], in_=w1q[m, 0])
                tl = w1_pool.tile([P, KT1, 2, P], F8, tag="w1",
                                  name=f"w1l{m}")
                (eng or nc.sync).dma_start(out=tl[:], in_=w1q[m, 1])
                w1_tiles[m] = (th, tl)

            # Startup DMAs fanned over the THREE DMA-capable queues (each
            # DMA holds its queue >=500ns, so two queues cannot feed m=0's
            # cadence).  ACT is idle until the first eviction (~4.5us) and
            # carries early x tiles.  Completion ~= 500*(slot+1) + 1716:
            #   SP:   w1h0 xh3 xl1 w1h1 b1 w1l1 w1h2 w1l2 | loop prefetch
            #   ACT:  xh1  xh2 xl2 w1l0 b2
            #   Pool: xh0  xl0 xl3 | w2-stream...
            th0 = w1_pool.tile([P, KT1, 2, P], F8, tag="w1", name="w1h0")
            tl0 = w1_pool.tile([P, KT1, 2, P], F8, tag="w1", name="w1l0")
            w1_tiles[0] = (th0, tl0)
            b1_sb = const.tile([P, MT1], F32)
            b2_sb = const.tile([P, NT2], F32)
            th1 = w1_pool.tile([P, KT1, 2, P], F8, tag="w1", name="w1h1")
            tl1 = w1_pool.tile([P, KT1, 2, P], F8, tag="w1", name="w1l1")
            w1_tiles[1] = (th1, tl1)
            nc.sync.dma_start(out=th0[:], in_=w1q[0, 0])
            nc.scalar.dma_start(out=xt_sb[0][1][:], in_=xt8[0, 1])
            nc.gpsimd.dma_start(out=xt_sb[0][0][:], in_=xt8[0, 0])
            nc.sync.dma_start(out=xt_sb[0][3][:], in_=xt8[0, 3])
            nc.scalar.dma_start(out=xt_sb[0][2][:], in_=xt8[0, 2])
            nc.gpsimd.dma_start(out=xt_sb[1][0][:], in_=xt8[1, 0])
            nc.sync.dma_start(out=xt_sb[1][1][:], in_=xt8[1, 1])
            nc.scalar.dma_start(out=xt_sb[1][2][:], in_=xt8[1, 2])
            nc.gpsimd.dma_start(out=xt_sb[1][3][:], in_=xt8[1, 3])
            nc.sync.dma_start(out=th1[:], in_=w1q[1, 0])
            nc.scalar.dma_start(out=tl0[:], in_=w1q[0, 1])
            nc.sync.dma_start(out=b1_sb[:], in_=b1t[:])
            nc.scalar.dma_start(out=b2_sb[:], in_=b2t[:])
            nc.sync.dma_start(out=tl1[:], in_=w1q[1, 1])
            w1_load(2)
            # kt consumption order per stream, matched to arrival order.
            KORD = [[1, 0, 3, 2], [0, 1, 2, 3], [0, 1, 2, 3]]

            # Prime ACT/DVE with the bias-load DMA waits so later evicts
            # (which also wait on PE sems) stay within the per-instruction
            # sync-wait budget in walrus codegen.
            prime1 = const.tile([P, 1], F32)
            nc.scalar.activation(prime1[:], b1_sb[:, 0:1], AF.Copy)
            prime2 = const.tile([P, 1], F32)
            nc.vector.tensor_copy(prime2[:], b2_sb[:, 0:1])

            # Resident tiles: W2 hi/lo [128, 2s, 16kt, 2i, 1024] (8MB) and
            # hT hi/lo [128, 2s, 16kt, 2i, 512] (4MB).
            w2_sb = big_pool.tile([P, 2, KT2, 2, N_OUT], F8, tag="w2",
                                  name="w2_sb")
            ht_sb = big_pool.tile([P, 2, KT2, 2, BSH], F8, tag="ht",
                                  name="ht_sb")

            # ---- GEMM1: h = relu(x @ W1 + b1), 3-stream fp8 DoubleRow ----
            for m in range(MT1):
                if m + 3 < MT1:
                    w1_load(m + 3)
                # Pace the W2 stream at one (s, kt) slice per m-iter on the
                # Pool queue: hi slices on m=0..15, lo on m=16.. (only the
                # k-prefix the w2_lo stream reads).
                if m < KT2 + W2LO_KTS:
                    ws, wkt = (0, m) if m < KT2 else (1, m - KT2)
                    nc.gpsimd.dma_start(out=w2_sb[:, ws, wkt, :, :],
                                        in_=w2q[ws, wkt])

                ps = ps_pool.tile([P, BSH], F32, tag="ps", name="ps")
                th, tl = w1_tiles.pop(m)
                nmm = 0
                for si, (wt, xs) in enumerate(((th, 0), (th, 1), (tl, 0))):
                    for kt in (KORD[si] if m == 0 else range(KT1)):
                        nc.tensor.matmul(
                            ps[:],
                            wt[:, kt, :, :],
                            xt_sb[xs][kt][:],
                            start=(nmm == 0),
                            stop=(nmm == 3 * KT1 - 1),
                            perf_mode=DR,
                        )
                        nmm += 1

                kt2, pl = m // 2, m % 2
                h16 = h16_pool.tile([P, BSH], F16, tag="h16", name="h16")
                nc.scalar.activation(h16[:], ps[:], AF.Relu,
                                     bias=b1_sb[:, m:m + 1], scale=1.0)
                hhi = ht_sb[:, 0, kt2, pl, :]
                # Alternate the h_hi cast between ACT and DVE so neither
                # engine saturates the GEMM1 window.
                if m % 2 == 0:
                    nc.scalar.activation(hhi, h16[:], AF.Copy)
                else:
                    nc.vector.tensor_copy(hhi, h16[:])
                if kt2 < HLO_KTS:
                    nc.vector.scalar_tensor_tensor(
                        ht_sb[:, 1, kt2, pl, :], h16[:], 1.0, hhi,
                        OP.mult, OP.subtract,
                    )

            # ---- GEMM2: yT = (hT.T-tiled @ W2 slices) + b2, 3-stream ----
            # The last tile is split into batch sub-chunks so the
            # after-last-matmul tail is a small evict + store instead of a
            # full [128, 512] evict + 256KB DMA.
            chunks = []
            for t in range(NT2):
                if t == NT2 - 1:
                    for c0, cw in ((0, 256), (256, 128), (384, 64),
                                   (448, 64)):
                        chunks.append((t, c0, cw))
                else:
                    chunks.append((t, 0, BSH))
            g2mms = []
            for kt in range(KT2):
                g2mms.append((0, 0, kt))
                if kt < HLO_KTS:
                    g2mms.append((0, 1, kt))
                if kt < W2LO_KTS:
                    g2mms.append((1, 0, kt))
            for t, c0, cw in chunks:
                ps2 = ps_pool.tile([P, cw], F32, tag="ps", name="ps2")
                for nmm, (ws, hs, kt) in enumerate(g2mms):
                    nc.tensor.matmul(
                        ps2[:],
                        w2_sb[:, ws, kt, :, t * P:(t + 1) * P],
                        ht_sb[:, hs, kt, :, c0:c0 + cw],
                        start=(nmm == 0),
                        stop=(nmm == len(g2mms) - 1),
                        perf_mode=DR,
                    )
                y_sb = y_pool.tile([P, cw], F32, tag="y", name="y_sb")
                nc.vector.tensor_scalar(
                    y_sb[:], ps2[:], b2_sb[:, t:t + 1], SY,
                    OP.add, OP.mult,
                )
                # The final store goes on SP (shorter DMA init than Pool —
                # it is the makespan tail).
                eng = nc.sync if (t, c0) == (NT2 - 1, 448) else nc.gpsimd
                eng.dma_start(out=yt[t * P:(t + 1) * P, c0:c0 + cw],
                              in_=y_sb[:])
    nc.compile()
    return nc


def _split8(a):
    hi = a.astype(E4)
    lo = (a - hi.astype(np.float32)).astype(E4)
    return hi, lo


def _prep_shared(W1, b1, W2, b2):
    W1s = np.asarray(W1, np.float32) * SW1          # [1024, 4096]
    w1hi, w1lo = _split8(W1s)
    a = np.stack([w1hi, w1lo])                      # [2s, 1024, 4096]
    a = a.reshape(2, KT1, 2, P, MT1, P)             # [s, kt, i, p, m, c]
    w1q = np.ascontiguousarray(a.transpose(4, 0, 3, 1, 2, 5))

    W2s = np.asarray(W2, np.float32) * SW2          # [4096, 1024]
    w2hi, w2lo = _split8(W2s)
    b = np.stack([w2hi, w2lo])                      # [2s, 4096, 1024]
    b = b.reshape(2, KT2, 2, P, N_OUT)              # [s, kt, i, p, c]
    w2q = np.ascontiguousarray(b.transpose(0, 1, 3, 2, 4))

    b1t = np.ascontiguousarray(
        (np.asarray(b1, np.float32) * SW1).reshape(MT1, P).T)
    b2t = np.ascontiguousarray(
        (np.asarray(b2, np.float32) / SY).reshape(NT2, P).T)
    return w1q, b1t, w2q, b2t


def kernel(x, W1, b1, W2, b2):
    x = np.asarray(x, np.float32)
    w1q, b1t, w2q, b2t = _prep_shared(W1, b1, W2, b2)

    in_maps = []
    for i in range(N_CORES):
        xT = np.ascontiguousarray(x[i * BSH:(i + 1) * BSH, :].T)
        xhi, xlo = _split8(xT)                      # [1024, 512] each
        a = np.stack([xhi, xlo]).reshape(2, KT1, 2, P, BSH)
        xt8 = np.ascontiguousarray(a.transpose(0, 1, 3, 2, 4))
        in_maps.append(
            {"xt8": xt8, "w1q": w1q, "w2q": w2q, "b1t": b1t, "b2t": b2t}
        )

    nc = build_nc()
    res = run_bass_kernel_spmd(nc, in_maps, list(range(N_CORES)))
    y = np.concatenate(
        [np.asarray(res.results[i]["yt"], dtype=np.float32).T
         for i in range(N_CORES)],
        axis=0,
    )
    return np.ascontiguousarray(y)


if __name__ == "__main__":
    rng = np.random.default_rng(0)
    x = rng.standard_normal((B, N_IN), dtype=np.float32)
    W1 = rng.standard_normal((N_IN, N_HID), dtype=np.float32) / 32
    b1 = rng.standard_normal((N_HID,), dtype=np.float32) / 32
    W2 = rng.standard_normal((N_HID, N_OUT), dtype=np.float32) / 64
    b2 = rng.standard_normal((N_OUT,), dtype=np.float32) / 64
    y = kernel(x, W1, b1, W2, b2)
    h = np.maximum(x @ W1 + b1, 0)
    y_ref = h @ W2 + b2
    err = np.linalg.norm(y - y_ref) / np.linalg.norm(y_ref)
    print("rel_l2:", err)
